# revision 7
# baseline (speedup 1.0000x reference)
"""2-layer GRU (B=64, T=512, D=512, H=1024) on 8 TRN2 NeuronCores.

Strategy: data-parallel over batch (8 sequences per core, GRU weights
replicated), per the time-sequential recurrence structure.

Per core, per layer:
  - input-side gates gi = x @ W_ih^T + b_ih precomputed as one large GEMM
    (time-independent), spilled to DRAM, streamed back per 16-step chunk.
  - the time recurrence runs h-stationary on the TensorEngine: per step,
    lhsT = h^T k-tiles [128,8] (fp32r), moving rhs = W_hh^T [128,512]
    slices; 8 k-tiles x 6 n-tiles accumulate gh = h @ W_hh^T in PSUM.
  - gates/h-update are split in two column halves so DVE/ACT work for one
    half hides under the PE's weight streaming of the other half; the
    per-step h -> h^T transpose runs on the PE (8 [8,128] transposes, split
    4+4 so each half's transpose interleaves with the next step's k0-3
    matmuls, keeping the PE bubble-free).

Matmuls use fp32r (full-rate fp32 storage, ~1.6e-4 matmul rel-err);
gates/updates are exact fp32 on DVE/ACT.
"""

import os
import numpy as np

B, T, D, H, L = 64, 512, 512, 1024, 2
G = 3 * H
N_CORES = 8
BL = B // N_CORES  # 8 sequences per core
CHUNK = 2  # recurrence steps per gi chunk (gi chunk tile [8, CHUNK*G])

_cache = {}


def _build(t_steps):
    import concourse.bacc as bacc
    import concourse.mybir as mybir
    from concourse import tile

    f32 = mybir.dt.float32
    f32r = mybir.dt.float32r
    SIG = mybir.ActivationFunctionType.Sigmoid
    TANH = mybir.ActivationFunctionType.Tanh

    n_chunks = t_steps // CHUNK
    rows = T * BL  # 4096 gi rows per core (full-T GEMMs regardless of t_steps)

    nc = bacc.Bacc("TRN2", target_bir_lowering=False, debug=False,
                   num_devices=N_CORES)

    # ---- external I/O (per-core shards / replicated weights) ----
    xT_d = nc.dram_tensor("xT", [D, rows], f32r, kind="ExternalInput").ap()
    h0g_d = nc.dram_tensor("h0g", [L * BL, H], f32, kind="ExternalInput").ap()
    h0T_d = nc.dram_tensor("h0T", [128, L * 64], f32r, kind="ExternalInput").ap()
    wih_d = [
        nc.dram_tensor("wih0", [128, (D // 128) * G], f32r, kind="ExternalInput").ap(),
        nc.dram_tensor("wih1", [128, (H // 128) * G], f32r, kind="ExternalInput").ap(),
    ]
    whh_d = [
        nc.dram_tensor("whh0", [128, 8 * G], f32r, kind="ExternalInput").ap(),
        nc.dram_tensor("whh1", [128, 8 * G], f32r, kind="ExternalInput").ap(),
    ]
    eye_d = nc.dram_tensor("eye8", [8, 8], f32, kind="ExternalInput").ap()
    eyer_d = nc.dram_tensor("eye8r", [8, 8], f32r, kind="ExternalInput").ap()
    out1_d = nc.dram_tensor("out1", [rows, H], f32, kind="ExternalOutput").ap()
    hn_d = nc.dram_tensor("hn", [L * BL, H], f32, kind="ExternalOutput").ap()

    with tile.TileContext(nc) as tc:
        with (
            tc.tile_pool(name="dram", bufs=1, space="DRAM") as dpool,
            tc.tile_pool(name="const", bufs=1) as cpool,
        ):
            gi_rz_d = [
                dpool.tile([rows, 2 * H], f32r, name="gi_rz0"),
                dpool.tile([rows, 2 * H], f32r, name="gi_rz1"),
            ]
            gi_n_d = [
                dpool.tile([rows, H], f32, name="gi_n0"),
                dpool.tile([rows, H], f32, name="gi_n1"),
            ]
            o0T_d = dpool.tile([H, rows], f32r, name="o0T")
            eye_s = cpool.tile([8, 8], f32, name="eye_s")
            nc.sync.dma_start(out=eye_s[:], in_=eye_d[:])
            eyer_s = cpool.tile([8, 8], f32r, name="eyer_s")
            nc.sync.dma_start(out=eyer_s[:], in_=eyer_d[:])

            # ---------------- input-gate GEMM:  gi = x @ W_ih^T ----------------
            def gi_gemm(src_d, kt, w_dram, rz_out, n_out):
                """src_d: [kt*128, rows] fp32r lhsT source."""
                with (
                    tc.tile_pool(name="wI", bufs=1) as wp,
                    tc.tile_pool(name="xt", bufs=3) as xp,
                    tc.tile_pool(name="ev", bufs=2) as ep,
                    tc.tile_pool(name="psg", bufs=1, space="PSUM") as pp,
                ):
                    w_s = wp.tile([128, kt * G], f32r, name="w_s")
                    nc.sync.dma_start(out=w_s[:], in_=w_dram[:])
                    psg = [pp.tile([128, 512], f32, name=f"psg{n}") for n in range(6)]
                    for rb in range(rows // 128):
                        xt = xp.tile([128, kt * 128], f32r, name="xt")
                        nc.sync.dma_start(
                            out=xt[:].rearrange("p (k r) -> p k r", k=kt),
                            in_=src_d[:, rb * 128:(rb + 1) * 128].rearrange(
                                "(k p) r -> p k r", p=128),
                        )
                        for k in range(kt):
                            for n in range(6):
                                nc.tensor.matmul(
                                    psg[n][:, :],
                                    xt[:, k * 128:(k + 1) * 128],
                                    w_s[:, k * G + n * 512: k * G + n * 512 + 512],
                                    start=(k == 0),
                                    stop=(k == kt - 1),
                                )
                        evrz = ep.tile([128, 2 * H], f32r, name="evrz")
                        evn = ep.tile([128, H], f32, name="evn")
                        for n in range(4):
                            nc.vector.tensor_copy(
                                evrz[:, n * 512:(n + 1) * 512], psg[n][:, :])
                        for n in range(4, 6):
                            nc.vector.tensor_copy(
                                evn[:, (n - 4) * 512:(n - 3) * 512], psg[n][:, :])
                        nc.sync.dma_start(
                            out=rz_out[rb * 128:(rb + 1) * 128, :], in_=evrz[:])
                        nc.sync.dma_start(
                            out=n_out[rb * 128:(rb + 1) * 128, :], in_=evn[:])

            # ---------------- recurrence for one layer ----------------
            def recurrence(layer):
                rz_l = gi_rz_d[layer]
                n_l = gi_n_d[layer]
                with (
                    tc.tile_pool(name="wR", bufs=1) as wp,
                    tc.tile_pool(name="gic", bufs=1) as gp,
                    tc.tile_pool(name="st", bufs=1) as sp,
                    tc.tile_pool(name="gt", bufs=1) as tp,
                    tc.tile_pool(name="psR", bufs=1, space="PSUM") as pp,
                ):
                    w_s = wp.tile([128, 8 * G], f32r, name="w_s")
                    nc.sync.dma_start(out=w_s[:], in_=whh_d[layer][:])
                    gicr = [gp.tile([BL, CHUNK * 2 * H], f32r, name=f"gicr{i}")
                            for i in range(2)]
                    gicn = [gp.tile([BL, CHUNK * H], f32, name=f"gicn{i}")
                            for i in range(2)]
                    h_t = [sp.tile([BL, H], f32, name=f"h{i}") for i in range(2)]
                    hT_t = [sp.tile([128, 64], f32r, name=f"hT{i}") for i in range(2)]
                    # psum: 6 gate banks + 2 transpose banks = 8
                    ps = {}
                    for half in range(2):
                        for nm in ("r", "z", "n"):
                            ps[(nm, half)] = pp.tile([BL, 512], f32,
                                                     name=f"ps_{nm}{half}")
                    ptr = [pp.tile([128, 32], f32, name=f"ptr{i}") for i in range(2)]

                    nc.sync.dma_start(out=h_t[0][:],
                                      in_=h0g_d[layer * BL:(layer + 1) * BL, :])
                    nc.sync.dma_start(out=hT_t[0][:],
                                      in_=h0T_d[:, layer * 64:(layer + 1) * 64])
                    nc.sync.dma_start(
                        out=gicr[0][:].rearrange("b (j g) -> b j g", j=CHUNK),
                        in_=rz_l[0:CHUNK * BL, :].rearrange(
                            "(j b) g -> b j g", b=BL))
                    nc.sync.dma_start(
                        out=gicn[0][:].rearrange("b (j g) -> b j g", j=CHUNK),
                        in_=n_l[0:CHUNK * BL, :].rearrange(
                            "(j b) g -> b j g", b=BL))

                    # column starts in W^T / gh for (gate, half)
                    col = {("r", 0): 0, ("r", 1): 512,
                           ("z", 0): H, ("z", 1): H + 512,
                           ("n", 0): 2 * H, ("n", 1): 2 * H + 512}

                    def inject(t, half):
                        """Open the r/z accumulation groups of step t with the
                        precomputed input-side gates: ps = I8 @ gi (start=True),
                        so the sigmoids read PSUM directly (no DVE adds on the
                        serial gate chain)."""
                        j = t % CHUNK
                        cs = half * 512
                        gr = gicr[(t // CHUNK) % 2]
                        base = j * 2 * H
                        nc.tensor.matmul(
                            ps[("r", half)][:, :], eyer_s[:],
                            gr[:, base + cs:base + cs + 512],
                            start=True, stop=False)

                    def mm_phase(t, ks, half):
                        """k-tiles ks of step t into this half's three psums."""
                        hT = hT_t[t % 2]
                        for k in ks:
                            for nm in ("r", "z", "n"):
                                nc.tensor.matmul(
                                    ps[(nm, half)][:, :],
                                    hT[:, k * 8:(k + 1) * 8],
                                    w_s[:, k * G + col[(nm, half)]:
                                        k * G + col[(nm, half)] + 512],
                                    start=(k == 0 and nm != "r"),
                                    stop=(k == 7),
                                )

                    def gates(t, half):
                        j = t % CHUNK
                        cs = half * 512
                        gn = gicn[(t // CHUNK) % 2]
                        hc = h_t[t % 2]
                        hn = h_t[(t + 1) % 2]
                        rz = tp.tile([BL, H], f32, name=f"rz{half}")
                        t1 = tp.tile([BL, 512], f32, name=f"t1{half}")
                        t2 = tp.tile([BL, 512], f32, name=f"t2{half}")
                        nn_ = tp.tile([BL, 512], f32, name=f"nn{half}")
                        d_ = tp.tile([BL, 512], f32, name=f"d{half}")
                        e_ = tp.tile([BL, 512], f32, name=f"e{half}")
                        gr_ = gicr[(t // CHUNK) % 2]
                        gz_ = tp.tile([BL, 512], f32, name=f"gz{half}")
                        nc.scalar.activation(rz[:, 0:512], ps[("r", half)][:, :],
                                             SIG)
                        nc.vector.tensor_mul(t1[:], rz[:, 0:512],
                                             ps[("n", half)][:, :])
                        nc.vector.tensor_add(
                            gz_[:], ps[("z", half)][:, :],
                            gr_[:, j * 2 * H + H + cs:j * 2 * H + H + cs + 512]
                            .bitcast(f32))
                        nc.scalar.activation(rz[:, 512:H], gz_[:], SIG)
                        nc.vector.tensor_add(t2[:], t1[:],
                                             gn[:, j * H + cs:j * H + cs + 512])
                        nc.scalar.activation(nn_[:], t2[:], TANH)
                        nc.vector.tensor_sub(d_[:], nn_[:], hc[:, cs:cs + 512])
                        nc.vector.tensor_mul(e_[:], rz[:, 512:H], d_[:])
                        nc.vector.tensor_add(hn[:, cs:cs + 512],
                                             hc[:, cs:cs + 512], e_[:])

                    def transpose_half(t, half):
                        """h_new cols [half*512, half*512+512) -> hT_new k-tiles."""
                        hn = h_t[(t + 1) % 2]
                        hTn = hT_t[(t + 1) % 2]
                        for qq in range(4):
                            q = half * 4 + qq
                            nc.tensor.transpose(
                                ptr[half][:, qq * 8:(qq + 1) * 8],
                                hn[:, q * 128:(q + 1) * 128],
                                eye_s[:],
                            )
                            nc.vector.tensor_copy(
                                hTn[:, q * 8:(q + 1) * 8],
                                ptr[half][:, qq * 8:(qq + 1) * 8])

                    def out_dma(t):
                        hn = h_t[(t + 1) % 2]
                        hTn = hT_t[(t + 1) % 2]
                        if layer == 0:
                            nc.sync.dma_start(
                                out=o0T_d[:, t * BL:(t + 1) * BL].rearrange(
                                    "(k p) b -> p k b", p=128),
                                in_=hTn[:, :].rearrange("p (k b) -> p k b", b=8),
                            )
                        else:
                            nc.sync.dma_start(
                                out=out1_d[t * BL:(t + 1) * BL, :], in_=hn[:])

                    # prologue: injects + k0-1 matmuls of step 0
                    inject(0, 0)
                    inject(0, 1)
                    mm_phase(0, range(0, 2), 0)
                    mm_phase(0, range(0, 2), 1)
                    for t in range(t_steps):
                        j = t % CHUNK
                        c = t // CHUNK
                        if j == 0 and c + 1 < n_chunks:
                            nc.sync.dma_start(
                                out=gicr[(c + 1) % 2][:].rearrange(
                                    "b (j g) -> b j g", j=CHUNK),
                                in_=rz_l[(c + 1) * CHUNK * BL:
                                         (c + 2) * CHUNK * BL, :].rearrange(
                                    "(j b) g -> b j g", b=BL))
                            nc.sync.dma_start(
                                out=gicn[(c + 1) % 2][:].rearrange(
                                    "b (j g) -> b j g", j=CHUNK),
                                in_=n_l[(c + 1) * CHUNK * BL:
                                        (c + 2) * CHUNK * BL, :].rearrange(
                                    "(j b) g -> b j g", b=BL))
                        mm_phase(t, range(2, 8), 0)
                        mm_phase(t, range(2, 8), 1)
                        gates(t, 0)
                        last = t == t_steps - 1
                        if not last:
                            inject(t + 1, 0)
                            transpose_half(t, 0)
                            mm_phase(t + 1, range(0, 2), 0)
                        gates(t, 1)
                        if not last:
                            inject(t + 1, 1)
                            mm_phase(t + 1, range(0, 2), 1)
                            transpose_half(t, 1)
                        else:
                            # layer output still needs h^T for o0T (layer 0)
                            if layer == 0:
                                transpose_half(t, 0)
                                transpose_half(t, 1)
                        out_dma(t)
                    nc.sync.dma_start(
                        out=hn_d[layer * BL:(layer + 1) * BL, :],
                        in_=h_t[t_steps % 2][:])

            gi_gemm(xT_d, D // 128, wih_d[0], gi_rz_d[0], gi_n_d[0])
            recurrence(0)
            gi_gemm(o0T_d, H // 128, wih_d[1], gi_rz_d[1], gi_n_d[1])
            recurrence(1)

    nc.compile()
    return nc


def _get_program(t_steps):
    if t_steps not in _cache:
        _cache[t_steps] = _build(t_steps)
    return _cache[t_steps]


def kernel(x, h0, W_ih0, W_hh0, b_ih0, b_hh0, W_ih1, W_hh1, b_ih1, b_hh1):
    from concourse.bass_utils import run_bass_kernel_spmd

    t_steps = int(os.environ.get("GRU_T_STEPS", T))
    x = np.asarray(x, np.float32)
    h0 = np.asarray(h0, np.float32)
    assert not (np.any(b_ih0) or np.any(b_hh0) or np.any(b_ih1)
                or np.any(b_hh1)), "nonzero biases not supported"

    def ktile(w):  # [G, K] -> [128, (K//128)*G] fp32 k-tile layout of W^T
        K = w.shape[1]
        return np.ascontiguousarray(
            np.asarray(w, np.float32).T.reshape(K // 128, 128, G)
            .transpose(1, 0, 2).reshape(128, (K // 128) * G))

    wih = [ktile(W_ih0), ktile(W_ih1)]
    whh = [ktile(W_hh0), ktile(W_hh1)]
    eye8 = np.eye(8, dtype=np.float32)

    in_maps = []
    for c in range(N_CORES):
        xl = np.asarray(x[c * BL:(c + 1) * BL], np.float32)  # [8,T,D]
        xT = np.ascontiguousarray(xl.transpose(2, 1, 0).reshape(D, T * BL))
        h0l = np.asarray(h0[:, c * BL:(c + 1) * BL], np.float32)  # [2,8,H]
        h0g = np.ascontiguousarray(h0l.reshape(L * BL, H))
        h0T = np.ascontiguousarray(
            h0l.reshape(L, BL, H // 128, 128).transpose(3, 0, 2, 1)
            .reshape(128, L * 64))
        # h0T[p, l*64 + k*8 + b] = h0[l, b, k*128+p]
        in_maps.append({
            "xT": xT, "h0g": h0g, "h0T": h0T,
            "wih0": wih[0], "wih1": wih[1],
            "whh0": whh[0], "whh1": whh[1],
            "eye8": eye8, "eye8r": eye8,
        })

    nc = _get_program(t_steps)
    res = run_bass_kernel_spmd(nc, in_maps, core_ids=list(range(N_CORES)),
                               trace=bool(int(os.environ.get("GRU_TRACE", "0"))))
    kernel.last_exec_time_ns = res.exec_time_ns

    out1 = np.empty((B, T, H), np.float32)
    h_n = np.empty((L, B, H), np.float32)
    for c in range(N_CORES):
        r = res.results[c]
        out1[c * BL:(c + 1) * BL] = (
            r["out1"].reshape(T, BL, H).transpose(1, 0, 2))
        h_n[:, c * BL:(c + 1) * BL] = r["hn"].reshape(L, BL, H)
    return out1, h_n


# revision 8
# speedup vs baseline: 1.0762x; 1.0762x over previous
"""2-layer GRU (B=64, T=512, D=512, H=1024) on 8 TRN2 NeuronCores.

Strategy: data-parallel over batch (8 sequences per core, GRU weights
replicated), per the time-sequential recurrence structure.

Per core, per layer:
  - input-side gates gi = x @ W_ih^T + b_ih precomputed as one large GEMM
    (time-independent), spilled to DRAM, streamed back per 16-step chunk.
  - the time recurrence runs h-stationary on the TensorEngine: per step,
    lhsT = h^T k-tiles [128,8] (fp32r), moving rhs = W_hh^T [128,512]
    slices; 8 k-tiles x 6 n-tiles accumulate gh = h @ W_hh^T in PSUM.
  - gates/h-update are split in two column halves so DVE/ACT work for one
    half hides under the PE's weight streaming of the other half; the
    per-step h -> h^T transpose runs on the PE (8 [8,128] transposes, split
    4+4 so each half's transpose interleaves with the next step's k0-3
    matmuls, keeping the PE bubble-free).

Matmuls use fp32r (full-rate fp32 storage, ~1.6e-4 matmul rel-err);
gates/updates are exact fp32 on DVE/ACT.
"""

import os
import numpy as np

B, T, D, H, L = 64, 512, 512, 1024, 2
G = 3 * H
N_CORES = 8
BL = B // N_CORES  # 8 sequences per core
CHUNK = 2  # recurrence steps per gi chunk (gi chunk tile [8, CHUNK*G])

_cache = {}


def _build(t_steps):
    import concourse.bacc as bacc
    import concourse.mybir as mybir
    from concourse import tile

    f32 = mybir.dt.float32
    f32r = mybir.dt.float32r
    SIG = mybir.ActivationFunctionType.Sigmoid
    TANH = mybir.ActivationFunctionType.Tanh

    n_chunks = t_steps // CHUNK
    rows = T * BL  # 4096 gi rows per core (full-T GEMMs regardless of t_steps)

    nc = bacc.Bacc("TRN2", target_bir_lowering=False, debug=False,
                   num_devices=N_CORES)

    # ---- external I/O (per-core shards / replicated weights) ----
    xT_d = nc.dram_tensor("xT", [D, rows], f32r, kind="ExternalInput").ap()
    h0g_d = nc.dram_tensor("h0g", [L * BL, H], f32, kind="ExternalInput").ap()
    h0T_d = nc.dram_tensor("h0T", [128, L * 64], f32r, kind="ExternalInput").ap()
    wih_d = [
        nc.dram_tensor("wih0", [128, (D // 128) * G], f32r, kind="ExternalInput").ap(),
        nc.dram_tensor("wih1", [128, (H // 128) * G], f32r, kind="ExternalInput").ap(),
    ]
    whh_d = [
        nc.dram_tensor("whh0", [128, 8 * G], f32r, kind="ExternalInput").ap(),
        nc.dram_tensor("whh1", [128, 8 * G], f32r, kind="ExternalInput").ap(),
    ]
    eye_d = nc.dram_tensor("eye8", [8, 8], f32, kind="ExternalInput").ap()
    eyer_d = nc.dram_tensor("eye8r", [8, 8], f32r, kind="ExternalInput").ap()
    out1_d = nc.dram_tensor("out1", [rows, H], f32, kind="ExternalOutput").ap()
    hn_d = nc.dram_tensor("hn", [L * BL, H], f32, kind="ExternalOutput").ap()

    with tile.TileContext(nc) as tc:
        with (
            tc.tile_pool(name="dram", bufs=1, space="DRAM") as dpool,
            tc.tile_pool(name="const", bufs=1) as cpool,
        ):
            gi_rz_d = [
                dpool.tile([rows, 2 * H], f32r, name="gi_rz0"),
                dpool.tile([rows, 2 * H], f32r, name="gi_rz1"),
            ]
            gi_n_d = [
                dpool.tile([rows, H], f32, name="gi_n0"),
                dpool.tile([rows, H], f32, name="gi_n1"),
            ]
            o0T_d = dpool.tile([H, rows], f32r, name="o0T")
            eye_s = cpool.tile([8, 8], f32, name="eye_s")
            nc.sync.dma_start(out=eye_s[:], in_=eye_d[:])
            eyer_s = cpool.tile([8, 8], f32r, name="eyer_s")
            nc.sync.dma_start(out=eyer_s[:], in_=eyer_d[:])

            # ---------------- input-gate GEMM:  gi = x @ W_ih^T ----------------
            def gi_gemm(src_d, kt, w_dram, rz_out, n_out):
                """src_d: [kt*128, rows] fp32r lhsT source."""
                with (
                    tc.tile_pool(name="wI", bufs=1) as wp,
                    tc.tile_pool(name="xt", bufs=3) as xp,
                    tc.tile_pool(name="ev", bufs=2) as ep,
                    tc.tile_pool(name="psg", bufs=1, space="PSUM") as pp,
                ):
                    w_s = wp.tile([128, kt * G], f32r, name="w_s")
                    nc.sync.dma_start(out=w_s[:], in_=w_dram[:])
                    psg = [pp.tile([128, 512], f32, name=f"psg{n}") for n in range(6)]
                    for rb in range(rows // 128):
                        xt = xp.tile([128, kt * 128], f32r, name="xt")
                        nc.sync.dma_start(
                            out=xt[:].rearrange("p (k r) -> p k r", k=kt),
                            in_=src_d[:, rb * 128:(rb + 1) * 128].rearrange(
                                "(k p) r -> p k r", p=128),
                        )
                        for k in range(kt):
                            for n in range(6):
                                nc.tensor.matmul(
                                    psg[n][:, :],
                                    xt[:, k * 128:(k + 1) * 128],
                                    w_s[:, k * G + n * 512: k * G + n * 512 + 512],
                                    start=(k == 0),
                                    stop=(k == kt - 1),
                                )
                        evrz = ep.tile([128, 2 * H], f32r, name="evrz")
                        evn = ep.tile([128, H], f32, name="evn")
                        for n in range(4):
                            nc.vector.tensor_copy(
                                evrz[:, n * 512:(n + 1) * 512], psg[n][:, :])
                        for n in range(4, 6):
                            nc.vector.tensor_copy(
                                evn[:, (n - 4) * 512:(n - 3) * 512], psg[n][:, :])
                        nc.sync.dma_start(
                            out=rz_out[rb * 128:(rb + 1) * 128, :], in_=evrz[:])
                        nc.sync.dma_start(
                            out=n_out[rb * 128:(rb + 1) * 128, :], in_=evn[:])

            # ---------------- recurrence for one layer ----------------
            def recurrence(layer):
                rz_l = gi_rz_d[layer]
                n_l = gi_n_d[layer]
                with (
                    tc.tile_pool(name="wR", bufs=1) as wp,
                    tc.tile_pool(name="gic", bufs=1) as gp,
                    tc.tile_pool(name="st", bufs=1) as sp,
                    tc.tile_pool(name="gt", bufs=1) as tp,
                    tc.tile_pool(name="psR", bufs=1, space="PSUM") as pp,
                ):
                    w_s = wp.tile([128, 8 * G], f32r, name="w_s")
                    nc.sync.dma_start(out=w_s[:], in_=whh_d[layer][:])
                    gicr = [gp.tile([BL, CHUNK * 2 * H], f32r, name=f"gicr{i}")
                            for i in range(2)]
                    gicn = [gp.tile([BL, CHUNK * H], f32, name=f"gicn{i}")
                            for i in range(2)]
                    h_t = [sp.tile([BL, H], f32, name=f"h{i}") for i in range(2)]
                    hT_t = [sp.tile([128, 64], f32r, name=f"hT{i}") for i in range(2)]
                    # psum: 6 gate banks + 2 transpose banks = 8
                    ps = {}
                    for half in range(2):
                        for nm in ("r", "z", "n"):
                            ps[(nm, half)] = pp.tile([BL, 512], f32,
                                                     name=f"ps_{nm}{half}")
                    ptr = [pp.tile([128, 32], f32, name=f"ptr{i}") for i in range(2)]

                    nc.sync.dma_start(out=h_t[0][:],
                                      in_=h0g_d[layer * BL:(layer + 1) * BL, :])
                    nc.sync.dma_start(out=hT_t[0][:],
                                      in_=h0T_d[:, layer * 64:(layer + 1) * 64])
                    nc.sync.dma_start(
                        out=gicr[0][:].rearrange("b (j g) -> b j g", j=CHUNK),
                        in_=rz_l[0:CHUNK * BL, :].rearrange(
                            "(j b) g -> b j g", b=BL))
                    nc.sync.dma_start(
                        out=gicn[0][:].rearrange("b (j g) -> b j g", j=CHUNK),
                        in_=n_l[0:CHUNK * BL, :].rearrange(
                            "(j b) g -> b j g", b=BL))

                    # column starts in W^T / gh for (gate, half)
                    col = {("r", 0): 0, ("r", 1): 512,
                           ("z", 0): H, ("z", 1): H + 512,
                           ("n", 0): 2 * H, ("n", 1): 2 * H + 512}

                    def inject(t, half):
                        """Open the r/z accumulation groups of step t with the
                        precomputed input-side gates: ps = I8 @ gi (start=True),
                        so the sigmoids read PSUM directly (no DVE adds on the
                        serial gate chain)."""
                        j = t % CHUNK
                        cs = half * 512
                        gr = gicr[(t // CHUNK) % 2]
                        base = j * 2 * H
                        nc.tensor.matmul(
                            ps[("r", half)][:, :], eyer_s[:],
                            gr[:, base + cs:base + cs + 512],
                            start=True, stop=False)
                        nc.tensor.matmul(
                            ps[("z", half)][:, :], eyer_s[:],
                            gr[:, base + H + cs:base + H + cs + 512],
                            start=True, stop=False)

                    def mm_phase(t, ks, half):
                        """k-tiles ks of step t into this half's three psums."""
                        hT = hT_t[t % 2]
                        for k in ks:
                            for nm in ("r", "z", "n"):
                                nc.tensor.matmul(
                                    ps[(nm, half)][:, :],
                                    hT[:, k * 8:(k + 1) * 8],
                                    w_s[:, k * G + col[(nm, half)]:
                                        k * G + col[(nm, half)] + 512],
                                    start=(k == 0 and nm == "n"),
                                    stop=(k == 7),
                                )

                    def gates(t, half):
                        j = t % CHUNK
                        cs = half * 512
                        gn = gicn[(t // CHUNK) % 2]
                        hc = h_t[t % 2]
                        hn = h_t[(t + 1) % 2]
                        rz = tp.tile([BL, H], f32, name=f"rz{half}")
                        t1 = tp.tile([BL, 512], f32, name=f"t1{half}")
                        t2 = tp.tile([BL, 512], f32, name=f"t2{half}")
                        nn_ = tp.tile([BL, 512], f32, name=f"nn{half}")
                        d_ = tp.tile([BL, 512], f32, name=f"d{half}")
                        e_ = tp.tile([BL, 512], f32, name=f"e{half}")
                        nc.scalar.activation(rz[:, 0:512], ps[("r", half)][:, :],
                                             SIG)
                        nc.vector.tensor_mul(t1[:], rz[:, 0:512],
                                             ps[("n", half)][:, :])
                        nc.scalar.activation(rz[:, 512:H], ps[("z", half)][:, :],
                                             SIG)
                        nc.vector.tensor_add(t2[:], t1[:],
                                             gn[:, j * H + cs:j * H + cs + 512])
                        nc.scalar.activation(nn_[:], t2[:], TANH)
                        nc.vector.tensor_sub(d_[:], nn_[:], hc[:, cs:cs + 512])
                        nc.vector.tensor_mul(e_[:], rz[:, 512:H], d_[:])
                        nc.vector.tensor_add(hn[:, cs:cs + 512],
                                             hc[:, cs:cs + 512], e_[:])

                    def transpose_half(t, half):
                        """h_new cols [half*512, half*512+512) -> hT_new k-tiles."""
                        hn = h_t[(t + 1) % 2]
                        hTn = hT_t[(t + 1) % 2]
                        for qq in range(4):
                            q = half * 4 + qq
                            nc.tensor.transpose(
                                ptr[half][:, qq * 8:(qq + 1) * 8],
                                hn[:, q * 128:(q + 1) * 128],
                                eye_s[:],
                            )
                            nc.vector.tensor_copy(
                                hTn[:, q * 8:(q + 1) * 8],
                                ptr[half][:, qq * 8:(qq + 1) * 8])

                    def out_dma(t):
                        hn = h_t[(t + 1) % 2]
                        hTn = hT_t[(t + 1) % 2]
                        if layer == 0:
                            nc.sync.dma_start(
                                out=o0T_d[:, t * BL:(t + 1) * BL].rearrange(
                                    "(k p) b -> p k b", p=128),
                                in_=hTn[:, :].rearrange("p (k b) -> p k b", b=8),
                            )
                        else:
                            nc.sync.dma_start(
                                out=out1_d[t * BL:(t + 1) * BL, :], in_=hn[:])

                    # prologue: injects + k0-1 matmuls of step 0
                    inject(0, 0)
                    inject(0, 1)
                    mm_phase(0, range(0, 2), 0)
                    mm_phase(0, range(0, 2), 1)
                    for t in range(t_steps):
                        j = t % CHUNK
                        c = t // CHUNK
                        if j == 0 and c + 1 < n_chunks:
                            nc.sync.dma_start(
                                out=gicr[(c + 1) % 2][:].rearrange(
                                    "b (j g) -> b j g", j=CHUNK),
                                in_=rz_l[(c + 1) * CHUNK * BL:
                                         (c + 2) * CHUNK * BL, :].rearrange(
                                    "(j b) g -> b j g", b=BL))
                            nc.sync.dma_start(
                                out=gicn[(c + 1) % 2][:].rearrange(
                                    "b (j g) -> b j g", j=CHUNK),
                                in_=n_l[(c + 1) * CHUNK * BL:
                                        (c + 2) * CHUNK * BL, :].rearrange(
                                    "(j b) g -> b j g", b=BL))
                        mm_phase(t, range(2, 8), 0)
                        mm_phase(t, range(2, 8), 1)
                        gates(t, 0)
                        last = t == t_steps - 1
                        if not last:
                            inject(t + 1, 0)
                            transpose_half(t, 0)
                            mm_phase(t + 1, range(0, 2), 0)
                        gates(t, 1)
                        if not last:
                            inject(t + 1, 1)
                            mm_phase(t + 1, range(0, 2), 1)
                            transpose_half(t, 1)
                        else:
                            # layer output still needs h^T for o0T (layer 0)
                            if layer == 0:
                                transpose_half(t, 0)
                                transpose_half(t, 1)
                        out_dma(t)
                    nc.sync.dma_start(
                        out=hn_d[layer * BL:(layer + 1) * BL, :],
                        in_=h_t[t_steps % 2][:])

            gi_gemm(xT_d, D // 128, wih_d[0], gi_rz_d[0], gi_n_d[0])
            recurrence(0)
            gi_gemm(o0T_d, H // 128, wih_d[1], gi_rz_d[1], gi_n_d[1])
            recurrence(1)

    nc.compile()
    return nc


def _get_program(t_steps):
    if t_steps not in _cache:
        _cache[t_steps] = _build(t_steps)
    return _cache[t_steps]


def kernel(x, h0, W_ih0, W_hh0, b_ih0, b_hh0, W_ih1, W_hh1, b_ih1, b_hh1):
    from concourse.bass_utils import run_bass_kernel_spmd

    t_steps = int(os.environ.get("GRU_T_STEPS", T))
    x = np.asarray(x, np.float32)
    h0 = np.asarray(h0, np.float32)
    assert not (np.any(b_ih0) or np.any(b_hh0) or np.any(b_ih1)
                or np.any(b_hh1)), "nonzero biases not supported"

    def ktile(w):  # [G, K] -> [128, (K//128)*G] fp32 k-tile layout of W^T
        K = w.shape[1]
        return np.ascontiguousarray(
            np.asarray(w, np.float32).T.reshape(K // 128, 128, G)
            .transpose(1, 0, 2).reshape(128, (K // 128) * G))

    wih = [ktile(W_ih0), ktile(W_ih1)]
    whh = [ktile(W_hh0), ktile(W_hh1)]
    eye8 = np.eye(8, dtype=np.float32)

    in_maps = []
    for c in range(N_CORES):
        xl = np.asarray(x[c * BL:(c + 1) * BL], np.float32)  # [8,T,D]
        xT = np.ascontiguousarray(xl.transpose(2, 1, 0).reshape(D, T * BL))
        h0l = np.asarray(h0[:, c * BL:(c + 1) * BL], np.float32)  # [2,8,H]
        h0g = np.ascontiguousarray(h0l.reshape(L * BL, H))
        h0T = np.ascontiguousarray(
            h0l.reshape(L, BL, H // 128, 128).transpose(3, 0, 2, 1)
            .reshape(128, L * 64))
        # h0T[p, l*64 + k*8 + b] = h0[l, b, k*128+p]
        in_maps.append({
            "xT": xT, "h0g": h0g, "h0T": h0T,
            "wih0": wih[0], "wih1": wih[1],
            "whh0": whh[0], "whh1": whh[1],
            "eye8": eye8, "eye8r": eye8,
        })

    nc = _get_program(t_steps)
    res = run_bass_kernel_spmd(nc, in_maps, core_ids=list(range(N_CORES)),
                               trace=bool(int(os.environ.get("GRU_TRACE", "0"))))
    kernel.last_exec_time_ns = res.exec_time_ns

    out1 = np.empty((B, T, H), np.float32)
    h_n = np.empty((L, B, H), np.float32)
    for c in range(N_CORES):
        r = res.results[c]
        out1[c * BL:(c + 1) * BL] = (
            r["out1"].reshape(T, BL, H).transpose(1, 0, 2))
        h_n[:, c * BL:(c + 1) * BL] = r["hn"].reshape(L, BL, H)
    return out1, h_n
